# revision 24
# baseline (speedup 1.0000x reference)
"""Trainium2 Bass kernel for nn_DependencyBLSTM (gnn_message_passing).

Reference computation (per doc d, S=64 sentences):
    x      = E[word_indices[:, :, -1]]            # only the last word is used
    enc    = lrelu(x @ W_sent.T + b_sent)
    P      = lrelu(enc @ W_par.T + b_par)
    C      = lrelu(enc @ W_ch.T + b_ch)
    A      = softmax_over_i(mask_diag(P @ C.T))   # [S, S], softmax over axis i
    fri    = softmax(enc @ w_root + b_root)
    tmp    = A.T @ enc
    agg    = tmp + fri[:, None] * root_embed
    tmp3   = enc * A.sum(axis=1)[:, None]
    ri     = lrelu(concat([enc, agg, tmp3]) @ W_r.T + b_r)
    out    = ri.mean(axis=0) @ W_cls.T + b_cls
Returns (out [D,2], A [D,S,S], fri [D,S]).

Sharding: data-parallel over docs, 8 docs per core (512 sentences per core).
Embedding table replicated in device DRAM; rows gathered by indirect DMA.

On-chip layout (per core): feature dims (W=H=300) are chunked 3x100 on the
partition axis; the 512-sentence batch lives on the free axis (matches the
512-element fp32 PSUM bank / matmul moving-operand limit). The softmax over
parents (a partition-axis reduction) is done on the PE: ones-matmul column
sum, reciprocal on DVE, then a K=1 ones-outer-product matmul to broadcast
back across partitions.
"""

import numpy as np

D, S, L, W, H, V = 64, 64, 64, 300, 300, 50000
NCORES = 8
DPC = D // NCORES          # docs per core
SB = DPC * S               # sentence batch per core = 512
CK = 100                   # feature chunk size
NWC = W // CK              # 3 chunks over W
NHC = H // CK              # 3 chunks over H
NKR = 3 * H // CK          # 9 chunks over the concat feature dim

# b_all column map (packed per-partition constants, [CK, 22]):
#   0..2   w_root chunks          3..5   b_sent chunks
#   6..8   b_par chunks           9..11  b_ch chunks
#   12..14 b_r chunks             15..20 W_cls.T chunks ([CK,2] each)
#   21     b_cls (partitions 0..1)
BALL_COLS = 22

# Set by test.py for profiling; harness uses defaults.
TRACE = False
TRACE_KW = {}
LAST_RESULT = None
USE_F32R = True

_PROGRAM_CACHE = {}


def _build_program(use_f32r, debug_taps=False):
    import concourse.bacc as bacc
    import concourse.bass as bass
    import concourse.mybir as mybir
    import concourse.tile as tile

    f32 = mybir.dt.float32
    i32 = mybir.dt.int32
    AF = mybir.ActivationFunctionType
    X = mybir.AxisListType.X

    nc = bacc.Bacc("TRN2", target_bir_lowering=False, debug=False,
                   num_devices=NCORES)

    # --- DRAM I/O -------------------------------------------------------
    # fp32r (reduced-precision fp32) runs the big matmuls at 4x the fp32
    # rate when the moving dim is >= 256. Walrus requires every operand of
    # an fp32r matmul to be produced as fp32r, so weight DRAM tensors are
    # declared fp32r (bitwise-identical layout) and intermediate operand
    # tiles get dtype float32r with their producers casting on write.
    fmm = mybir.dt.float32r if use_f32r else f32
    NT = SB // 128  # gather tiles

    idx_d = nc.dram_tensor("idx", [128, SB // 128], i32, kind="ExternalInput")
    E_d = nc.dram_tensor("E", [V, W], f32, kind="ExternalInput")
    wsT_d = nc.dram_tensor("wsT", [W, H], fmm, kind="ExternalInput")
    wpT_d = nc.dram_tensor("wpT", [H, H], fmm, kind="ExternalInput")
    wcT_d = nc.dram_tensor("wcT", [H, H], fmm, kind="ExternalInput")
    wrT_d = nc.dram_tensor("wrT", [3 * H, H], fmm, kind="ExternalInput")
    wrc_d = nc.dram_tensor("wrootc", [CK, NHC], fmm, kind="ExternalInput")
    ball_d = nc.dram_tensor("ball", [CK, BALL_COLS], f32, kind="ExternalInput")
    root_d = nc.dram_tensor("rootr", [1, H], fmm, kind="ExternalInput")
    mask_d = nc.dram_tensor("mask", [S, SB], f32, kind="ExternalInput")
    aux_d = nc.dram_tensor("aux", [128, 258], f32, kind="ExternalInput")
    auxr_d = nc.dram_tensor("auxr", [1, 128], fmm, kind="ExternalInput")
    A_out = nc.dram_tensor("Aout", [S, SB], f32, kind="ExternalOutput")
    fri_out = nc.dram_tensor("friout", [DPC, S], f32, kind="ExternalOutput")
    cls_out = nc.dram_tensor("clsout", [2, DPC], f32, kind="ExternalOutput")
    taps = {}
    if debug_taps:
        for nm, shp in (("t_wr", [CK, NKR * H]), ("t_enco", [128, NT * H]),
                        ("t_sumrow", [1, SB]), ("t_agg", [CK, NHC * SB]),
                        ("t_tmp3", [CK, NHC * SB]), ("t_ri", [CK, NHC * SB]),
                        ("t_fin", [CK, NHC * DPC]), ("t_enc", [CK, NHC * SB]),
                        ("t_xg", [128, NT * W]), ("t_xT", [CK, NWC * SB])):
            taps[nm] = nc.dram_tensor(nm, shp, f32, kind="ExternalOutput")
        taps["t_idx"] = nc.dram_tensor("t_idx", [128, NT], i32, kind="ExternalOutput")

    with tile.TileContext(nc) as tc:
        with (
            tc.tile_pool(name="sb", bufs=1) as pool,
            tc.tile_pool(name="ps", bufs=7, space="PSUM") as psum,
            tc.tile_pool(name="wu", bufs=1, space="PSUM") as wupool,
        ):
            mm = nc.tensor.matmul

            # --- index load + embedding gather first (gpsimd is dedicated
            # to the gather; everything else goes over HWDGE) -------------
            idxt = pool.tile([128, NT], i32, tag="idxt")
            nc.sync.dma_start(idxt[:], idx_d[:])
            # NOTE: one indirect DMA per 128 rows — the HW SWDGE gather only
            # supports a single [128, 1] offset column per instruction (a
            # batched [128, NT] offset AP simulates fine but gathers garbage
            # on hardware).
            xg_all = pool.tile([128, NT, W], f32, tag="xg")
            for t in range(NT):
                nc.gpsimd.indirect_dma_start(
                    out=xg_all[:, t, :], out_offset=None, in_=E_d[:],
                    in_offset=bass.IndirectOffsetOnAxis(ap=idxt[:, t:t + 1], axis=0),
                )
            xg = [xg_all[:, t, :] for t in range(NT)]

            # --- constants / weights in (HWDGE), earliest-needed first --
            aux = pool.tile([128, 258], f32, tag="aux")
            nc.sync.dma_start(aux[:], aux_d[:])
            ident = aux[:, 0:128]
            ones_c = aux[0:S, 128:129]
            ones_r = aux[0:1, 129:257]
            ones_c128 = aux[:, 128:129]
            zrow = aux[0:1, 64:128]  # identity row 0, upper half: all zeros
            ws = pool.tile([CK, NWC, H], fmm, tag="ws")
            wp = pool.tile([CK, NHC, H], fmm, tag="wp")
            wc = pool.tile([CK, NHC, H], fmm, tag="wc")
            wr = pool.tile([CK, NKR, H], fmm, tag="wr")
            nc.sync.dma_start(ws[:], wsT_d[:].rearrange("(c p) h -> p c h", p=CK))
            ball = pool.tile([CK, BALL_COLS], f32, tag="ball")
            nc.sync.dma_start(ball[:], ball_d[:])
            wroot = pool.tile([CK, NHC], fmm, tag="wroot")
            nc.sync.dma_start(wroot[:], wrc_d[:])
            maskt = pool.tile([S, SB], f32, tag="maskt")
            nc.sync.dma_start(maskt[:], mask_d[:])
            nc.sync.dma_start(wp[:], wpT_d[:].rearrange("(c p) h -> p c h", p=CK))
            nc.sync.dma_start(wc[:], wcT_d[:].rearrange("(c p) h -> p c h", p=CK))
            rootr = pool.tile([1, H], fmm, tag="rootr")
            nc.sync.dma_start(rootr[:], root_d[:])
            nc.sync.dma_start(wr[:], wrT_d[:].rearrange("(c p) h -> p c h", p=CK))
            ones_rr = pool.tile([1, 128], fmm, tag="ones_rr")
            nc.sync.dma_start(ones_rr[:], auxr_d[:])

            # --- PE warm-up: ~3.5us of junk transposes while the gather
            # is in flight, so HAM reaches 2.4 GHz before the real matmuls.
            wut = wupool.tile([128, 128], f32, tag="wut")

            def pe_filler(n):
                # Real matmuls (transpose-mode doesn't register as PE-busy
                # for the HAM clock gate): keep the PE warm across known
                # sem-wait windows.
                for _ in range(n):
                    mm(wut[:], ident, ident)

            pe_filler(16)

            # --- xT[w, s_g] via PE transpose, tile-major so each gather
            # tile is consumed as soon as it lands --------------------------
            xT = pool.tile([CK, NWC, SB], fmm, tag="xT")
            xps = [psum.tile([CK, SB], f32, tag="ps", name=f"xp{i}") for i in range(NWC)]
            for t in range(NT):
                for w_i in range(NWC):
                    nc.tensor.transpose(
                        xps[w_i][:, 128 * t:128 * (t + 1)],
                        xg[t][:, CK * w_i:CK * (w_i + 1)], ident)
            for w_i in range(NWC):
                for t in range(NT):
                    nc.vector.tensor_copy(
                        xT[:, w_i, 128 * t:128 * (t + 1)],
                        xps[w_i][:, 128 * t:128 * (t + 1)])

            # --- encT = lrelu(W_sent @ x^T + b_sent), split into per-tile
            # column regions so the first regions start before the last
            # gather tile has arrived --------------------------------------
            encT = pool.tile([CK, NHC, SB], fmm, tag="encT")
            pes = [psum.tile([CK, SB], f32, tag="ps", name=f"pe{i}") for i in range(NHC)]
            for t in range(NT):
                sl = slice(128 * t, 128 * (t + 1))
                for hc in range(NHC):
                    for w_i in range(NWC):
                        mm(pes[hc][:, sl], ws[:, w_i, CK * hc:CK * (hc + 1)],
                           xT[:, w_i, sl],
                           start=(w_i == 0), stop=(w_i == NWC - 1))
            for hc in range(NHC):
                nc.scalar.activation(encT[:, hc, :], pes[hc][:], AF.Lrelu,
                                     bias=ball[:, 3 + hc:4 + hc], alpha=0.01)

            # --- P^T, C^T -----------------------------------------------
            PT = pool.tile([CK, NHC, SB], fmm, tag="PT")
            CT = pool.tile([CK, NHC, SB], fmm, tag="CT")
            for dst, wgt, bc in ((PT, wp, 6), (CT, wc, 9)):
                for hc in range(NHC):
                    pp = psum.tile([CK, SB], f32, tag="ps")
                    for kc in range(NHC):
                        mm(pp[:], wgt[:, kc, CK * hc:CK * (hc + 1)],
                           encT[:, kc, :],
                           start=(kc == 0), stop=(kc == NHC - 1))
                    nc.scalar.activation(dst[:, hc, :], pp[:], AF.Lrelu,
                                         bias=ball[:, bc + hc:bc + hc + 1],
                                         alpha=0.01)

            # --- root scores -> fri (early: needs only encT) ------------
            psc = psum.tile([1, SB], f32, tag="ps")
            for hc in range(NHC):
                mm(psc[:], wroot[:, hc:hc + 1], encT[:, hc, :],
                   start=(hc == 0), stop=(hc == NHC - 1))
            e_fri = pool.tile([1, SB], f32, tag="e_fri")
            nc.scalar.activation(e_fri[:], psc[:], AF.Exp)
            sfri = pool.tile([1, DPC], f32, tag="sfri")
            nc.vector.reduce_sum(sfri[:], e_fri[:].rearrange("p (d s) -> p d s", d=DPC),
                                 axis=X)
            rfri = pool.tile([1, DPC], f32, tag="rfri")
            nc.vector.reciprocal(rfri[:], sfri[:])
            fri_sb = pool.tile([1, SB], f32, tag="fri_sb")
            nc.vector.tensor_tensor(
                fri_sb[:].rearrange("p (d s) -> p d s", d=DPC),
                e_fri[:].rearrange("p (d s) -> p d s", d=DPC),
                rfri[:].rearrange("p (d o) -> p d o", o=1).to_broadcast([1, DPC, S]),
                op=mybir.AluOpType.mult)
            nc.sync.dma_start(fri_out[:], fri_sb[:])
            fri_r = pool.tile([1, SB], fmm, tag="fri_r")
            nc.vector.tensor_copy(fri_r[:], fri_sb[:])

            # --- riT accumulators: the encT contribution can run early,
            # filling PE gaps while the A stage's ACT/DVE work runs -------
            cat0 = []
            riT_ps = []
            for gc in range(NHC):
                pr = psum.tile([CK, SB], f32, tag="ps")
                riT_ps.append(pr)
                for kc in range(NHC):
                    mm(pr[:], wr[:, kc, CK * gc:CK * (gc + 1)], encT[:, kc, :],
                       start=(kc == 0), stop=False, skip_group_check=True)

            # --- enc in global [s_g, h] orientation (2 docs per tile) ---
            enc_g = pool.tile([128, NT, H], f32, tag="enc_g")
            for t in range(NT):
                ep = psum.tile([128, H], f32, tag="ps")
                for hc in range(NHC):
                    nc.tensor.transpose(
                        ep[:, CK * hc:CK * (hc + 1)],
                        encT[:, hc, 128 * t:128 * (t + 1)].bitcast(f32),
                        ident[:CK, :CK])
                nc.scalar.activation(enc_g[:, t, :], ep[:], AF.Copy)

            # Block-diagonal staging of A: Ablk[q, 128t+c] holds doc 2t
            # (rows 0:64) / doc 2t+1 (rows 64:128) so tmp becomes 4 dense
            # K=128 matmuls per h-chunk instead of 8 per-doc ones.
            Ablk = pool.tile([128, SB], f32, tag="Ablk")
            nc.vector.memset(Ablk[:], 0.0)

            # --- A: raw scores, exp, mask, column-normalize -------------
            pA = psum.tile([S, SB], f32, tag="ps")
            for d in range(DPC):
                sl = slice(S * d, S * (d + 1))
                for hc in range(NHC):
                    mm(pA[:, sl], PT[:, hc, sl], CT[:, hc, sl],
                       start=(hc == 0), stop=(hc == NHC - 1))
            pe_filler(8)
            e_raw = pool.tile([S, SB], f32, tag="e_raw")
            nc.scalar.activation(e_raw[:], pA[:], AF.Exp)
            e_m = pool.tile([S, SB], f32, tag="e_m")
            nc.vector.tensor_mul(e_m[:], e_raw[:], maskt[:])
            # Prefetch the odd-doc band (partition shift of 64) with the
            # *unnormalized* values so the DMA overlaps the sums/reciprocal;
            # the odd band is normalized in place below.
            Ablk_v = Ablk[:].rearrange("p (t c) -> p t c", t=NT)
            e_v = e_m[:].rearrange("p (t c) -> p t c", t=NT)
            nc.sync.dma_start(Ablk_v[S:128, :, S:128], e_v[:, :, S:128])
            psS = psum.tile([1, SB], f32, tag="ps")
            mm(psS[:], ones_c, e_m[:])
            recip = pool.tile([1, SB], f32, tag="recip")
            nc.vector.reciprocal_approx_fast(out=recip[:], in_=psS[:])
            psB = psum.tile([128, SB], f32, tag="ps")
            mm(psB[:], ones_r[:, 0:128], recip[:])
            pe_filler(8)
            psB_v = psB[:].rearrange("p (t c) -> p t c", t=NT)
            A_sb = pool.tile([S, SB], f32, tag="A_sb")
            nc.vector.tensor_mul(A_sb[:], e_m[:], psB[0:S, :])
            nc.sync.dma_start(A_out[:], A_sb[:])
            nc.vector.tensor_mul(Ablk_v[S:128, :, S:128],
                                 Ablk_v[S:128, :, S:128], psB_v[S:128, :, S:128])
            A_v = A_sb[:].rearrange("p (t c) -> p t c", t=NT)
            nc.vector.tensor_copy(Ablk_v[0:S, :, 0:S], A_v[:, :, 0:S])

            # --- row sums of A -> row vector in (d, i) order ------------
            sumA = pool.tile([S, DPC], f32, tag="sumA")
            nc.vector.reduce_sum(sumA[:], A_sb[:].rearrange("p (d j) -> p d j", d=DPC),
                                 axis=X)
            psT = psum.tile([DPC, S], f32, tag="ps")
            nc.tensor.transpose(psT[:], sumA[:], ident[:S, :S])
            sumAT = pool.tile([DPC, S], f32, tag="sumAT")
            nc.vector.tensor_copy(sumAT[:], psT[:])
            sumRow = pool.tile([1, SB], f32, tag="sumRow")
            nc.sync.dma_start(sumRow[:], sumAT[:])
            sumRow_r = pool.tile([1, SB], fmm, tag="sumRow_r")
            nc.vector.tensor_copy(sumRow_r[:], sumRow[:])

            # --- aggT = enc^T A (block-diag) + root x fri ---------------
            aggT = pool.tile([CK, NHC, SB], fmm, tag="aggT")
            for hc in range(NHC):
                pt = psum.tile([CK, SB], f32, tag="ps")
                # Outer product first: start=True sets has_written for the
                # whole bank, so the matmuls below accumulate onto it.
                mm(pt[:], rootr[:, CK * hc:CK * (hc + 1)], fri_r[:],
                   start=True, stop=False, skip_group_check=True)
                for t in range(NT):
                    sl = slice(128 * t, 128 * (t + 1))
                    mm(pt[:, sl], enc_g[:, t, CK * hc:CK * (hc + 1)], Ablk[:, sl],
                       start=False, stop=(t == NT - 1), skip_group_check=True)
                nc.scalar.activation(aggT[:, hc, :], pt[:], AF.Copy)

            # --- tmp3T = encT * broadcast(sumRow) -----------------------
            ps3 = psum.tile([CK, SB], f32, tag="ps")
            mm(ps3[:], ones_rr[:, :CK], sumRow_r[:])
            tmp3T = pool.tile([CK, NHC, SB], fmm, tag="tmp3T")
            for hc in range(NHC):
                nc.vector.tensor_mul(tmp3T[:, hc, :], encT[:, hc, :], ps3[:])

            pe_filler(6)

            # --- riT: remaining chunks, then lrelu on DVE (no ACT
            # table switch away from Exp) --------------------------------
            cat = [encT, aggT, tmp3T]
            riT = pool.tile([CK, NHC, SB], f32, tag="riT")
            for gc in range(NHC):
                pr = riT_ps[gc]
                for kc in range(NHC, NKR):
                    src = cat[kc // 3][:, kc % 3, :]
                    mm(pr[:], wr[:, kc, CK * gc:CK * (gc + 1)], src,
                       start=False, stop=(kc == NKR - 1), skip_group_check=True)
                nc.scalar.activation(riT[:, gc, :], pr[:], AF.Lrelu,
                                     bias=ball[:, 12 + gc:13 + gc], alpha=0.01)

            # --- final mean + classifier --------------------------------
            fin = pool.tile([CK, NHC, DPC], f32, tag="fin")
            for gc in range(NHC):
                nc.vector.reduce_sum(
                    fin[:, gc, :],
                    riT[:, gc, :].rearrange("p (d s) -> p d s", d=DPC), axis=X)
            pe_filler(8)
            pc = psum.tile([2, DPC], f32, tag="ps")
            for gc in range(NHC):
                mm(pc[:], ball[:, 15 + 2 * gc:17 + 2 * gc], fin[:, gc, :],
                   start=(gc == 0), stop=(gc == NHC - 1))
            cls_sb = pool.tile([2, DPC], f32, tag="cls_sb")
            nc.vector.tensor_scalar(
                out=cls_sb[:], in0=pc[:], scalar1=1.0 / S,
                scalar2=ball[0:2, 21:22],
                op0=mybir.AluOpType.mult, op1=mybir.AluOpType.add)
            nc.sync.dma_start(cls_out[:], cls_sb[:])
            if debug_taps:
                nc.sync.dma_start(taps["t_wr"][:], wr[:].bitcast(f32))
                nc.sync.dma_start(taps["t_enco"][:], enc_g[:].rearrange("p t h -> p (t h)"))
                nc.sync.dma_start(taps["t_sumrow"][:], sumRow[:])
                nc.sync.dma_start(taps["t_agg"][:], aggT[:].bitcast(f32))
                nc.sync.dma_start(taps["t_tmp3"][:], tmp3T[:].bitcast(f32))
                nc.sync.dma_start(taps["t_ri"][:], riT[:])
                nc.sync.dma_start(taps["t_fin"][:], fin[:])
                nc.sync.dma_start(taps["t_enc"][:], encT[:].bitcast(f32))
                nc.sync.dma_start(taps["t_xg"][:], xg_all[:])
                nc.sync.dma_start(taps["t_xT"][:], xT[:].bitcast(f32))
                nc.sync.dma_start(taps["t_idx"][:], idxt[:])

    nc.compile()
    return nc


def get_program(use_f32r=None, debug_taps=False):
    if use_f32r is None:
        use_f32r = USE_F32R
    key = (use_f32r, debug_taps)
    if key not in _PROGRAM_CACHE:
        _PROGRAM_CACHE[key] = _build_program(use_f32r, debug_taps)
    return _PROGRAM_CACHE[key]


def prep_inputs(word_indices, E, W_sent, b_sent, W_par, b_par, W_ch, b_ch,
                w_root, b_root, root_embed, W_r, b_r, W_cls, b_cls):
    """Shard + reformat the full inputs into per-core in_maps."""
    f = lambda a: np.ascontiguousarray(np.asarray(a), dtype=np.float32)
    idx_last = np.asarray(word_indices)[:, :, L - 1].astype(np.int32)  # [D, S]

    E = f(E)
    wsT = f(np.asarray(W_sent).T)
    wpT = f(np.asarray(W_par).T)
    wcT = f(np.asarray(W_ch).T)
    wrT = f(np.asarray(W_r).T)

    ball = np.zeros((CK, BALL_COLS), np.float32)
    w_root, b_sent, b_par, b_ch, b_r = map(f, (w_root, b_sent, b_par, b_ch, b_r))
    W_cls, b_cls = f(W_cls), f(b_cls)
    for c in range(NHC):
        sl = slice(CK * c, CK * (c + 1))
        ball[:, 0 + c] = w_root[sl]
        ball[:, 3 + c] = b_sent[sl]
        ball[:, 6 + c] = b_par[sl]
        ball[:, 9 + c] = b_ch[sl]
        ball[:, 12 + c] = b_r[sl]
        ball[:, 15 + 2 * c:17 + 2 * c] = W_cls[:, sl].T
    ball[0:2, 21] = b_cls

    rootr = f(root_embed).reshape(1, H)
    wrootc = np.stack([w_root[CK * c:CK * (c + 1)] for c in range(NHC)], axis=1)
    wrootc = np.ascontiguousarray(wrootc)

    mask = np.ones((S, SB), np.float32)
    for i in range(S):
        mask[i, i::S] = 0.0

    aux = np.zeros((128, 258), np.float32)
    aux[:, 0:128] = np.eye(128, dtype=np.float32)
    aux[:, 128] = 1.0
    aux[0, 129:257] = 1.0
    auxr = np.ones((1, 128), np.float32)

    in_maps = []
    for c in range(NCORES):
        flat = idx_last[DPC * c:DPC * (c + 1)].reshape(-1)        # (d, s) order
        idx_pm = flat.reshape(SB // 128, 128).T.copy()            # [128, NT]
        in_maps.append({
            "idx": idx_pm, "E": E, "wsT": wsT, "wpT": wpT, "wcT": wcT,
            "wrT": wrT, "wrootc": wrootc, "ball": ball, "rootr": rootr,
            "mask": mask, "aux": aux, "auxr": auxr,
        })
    return in_maps


def unshard(results):
    outs, As, fris = [], [], []
    for r in results:
        outs.append(r["clsout"].T)          # [DPC, 2]
        # Aout is [i, (d, j)] on device; reorder to [d, i, j]
        As.append(r["Aout"].reshape(S, DPC, S).transpose(1, 0, 2))
        fris.append(r["friout"])            # [DPC, S]
    return (np.concatenate(outs, axis=0).astype(np.float32),
            np.concatenate(As, axis=0).astype(np.float32),
            np.concatenate(fris, axis=0).astype(np.float32))


def kernel(**inputs):
    global LAST_RESULT
    from concourse.bass_utils import run_bass_kernel_spmd

    nc = get_program()
    in_maps = prep_inputs(**inputs)
    res = run_bass_kernel_spmd(nc, in_maps, core_ids=list(range(NCORES)),
                               trace=TRACE, **TRACE_KW)
    LAST_RESULT = res
    return unshard(res.results)


# revision 25
# speedup vs baseline: 1.0441x; 1.0441x over previous
"""Trainium2 Bass kernel for nn_DependencyBLSTM (gnn_message_passing).

Reference computation (per doc d, S=64 sentences):
    x      = E[word_indices[:, :, -1]]            # only the last word is used
    enc    = lrelu(x @ W_sent.T + b_sent)
    P      = lrelu(enc @ W_par.T + b_par)
    C      = lrelu(enc @ W_ch.T + b_ch)
    A      = softmax_over_i(mask_diag(P @ C.T))   # [S, S], softmax over axis i
    fri    = softmax(enc @ w_root + b_root)
    tmp    = A.T @ enc
    agg    = tmp + fri[:, None] * root_embed
    tmp3   = enc * A.sum(axis=1)[:, None]
    ri     = lrelu(concat([enc, agg, tmp3]) @ W_r.T + b_r)
    out    = ri.mean(axis=0) @ W_cls.T + b_cls
Returns (out [D,2], A [D,S,S], fri [D,S]).

Sharding: data-parallel over docs, 8 docs per core (512 sentences per core).
Embedding table replicated in device DRAM; rows gathered by indirect DMA.

On-chip layout (per core): feature dims (W=H=300) are chunked 3x100 on the
partition axis; the 512-sentence batch lives on the free axis (matches the
512-element fp32 PSUM bank / matmul moving-operand limit). The softmax over
parents (a partition-axis reduction) is done on the PE: ones-matmul column
sum, reciprocal on DVE, then a K=1 ones-outer-product matmul to broadcast
back across partitions.
"""

import numpy as np

D, S, L, W, H, V = 64, 64, 64, 300, 300, 50000
NCORES = 8
DPC = D // NCORES          # docs per core
SB = DPC * S               # sentence batch per core = 512
CK = 100                   # feature chunk size
NWC = W // CK              # 3 chunks over W
NHC = H // CK              # 3 chunks over H
NKR = 3 * H // CK          # 9 chunks over the concat feature dim

# b_all column map (packed per-partition constants, [CK, 22]):
#   0..2   w_root chunks          3..5   b_sent chunks
#   6..8   b_par chunks           9..11  b_ch chunks
#   12..14 b_r chunks             15..20 W_cls.T chunks ([CK,2] each)
#   21     b_cls (partitions 0..1)
BALL_COLS = 22

# Set by test.py for profiling; harness uses defaults.
TRACE = False
TRACE_KW = {}
LAST_RESULT = None
USE_F32R = True

_PROGRAM_CACHE = {}


def _build_program(use_f32r, debug_taps=False):
    import concourse.bacc as bacc
    import concourse.bass as bass
    import concourse.mybir as mybir
    import concourse.tile as tile

    f32 = mybir.dt.float32
    i32 = mybir.dt.int32
    AF = mybir.ActivationFunctionType
    X = mybir.AxisListType.X

    nc = bacc.Bacc("TRN2", target_bir_lowering=False, debug=False,
                   num_devices=NCORES)

    # --- DRAM I/O -------------------------------------------------------
    # fp32r (reduced-precision fp32) runs the big matmuls at 4x the fp32
    # rate when the moving dim is >= 256. Walrus requires every operand of
    # an fp32r matmul to be produced as fp32r, so weight DRAM tensors are
    # declared fp32r (bitwise-identical layout) and intermediate operand
    # tiles get dtype float32r with their producers casting on write.
    fmm = mybir.dt.float32r if use_f32r else f32
    NT = SB // 128  # gather tiles

    idx_d = nc.dram_tensor("idx", [128, SB // 128], i32, kind="ExternalInput")
    E_d = nc.dram_tensor("E", [V, W], f32, kind="ExternalInput")
    wsT_d = nc.dram_tensor("wsT", [W, H], fmm, kind="ExternalInput")
    wpT_d = nc.dram_tensor("wpT", [H, H], fmm, kind="ExternalInput")
    wcT_d = nc.dram_tensor("wcT", [H, H], fmm, kind="ExternalInput")
    wrT_d = nc.dram_tensor("wrT", [3 * H, H], fmm, kind="ExternalInput")
    wrc_d = nc.dram_tensor("wrootc", [CK, NHC], fmm, kind="ExternalInput")
    ball_d = nc.dram_tensor("ball", [CK, BALL_COLS], f32, kind="ExternalInput")
    root_d = nc.dram_tensor("rootr", [1, H], fmm, kind="ExternalInput")
    mask_d = nc.dram_tensor("mask", [S, SB], f32, kind="ExternalInput")
    aux_d = nc.dram_tensor("aux", [128, 258], f32, kind="ExternalInput")
    auxr_d = nc.dram_tensor("auxr", [1, 128], fmm, kind="ExternalInput")
    A_out = nc.dram_tensor("Aout", [S, SB], f32, kind="ExternalOutput")
    fri_out = nc.dram_tensor("friout", [DPC, S], f32, kind="ExternalOutput")
    cls_out = nc.dram_tensor("clsout", [2, DPC], f32, kind="ExternalOutput")
    taps = {}
    if debug_taps:
        for nm, shp in (("t_wr", [CK, NKR * H]), ("t_enco", [128, NT * H]),
                        ("t_sumrow", [1, SB]), ("t_agg", [CK, NHC * SB]),
                        ("t_tmp3", [CK, NHC * SB]), ("t_ri", [CK, NHC * SB]),
                        ("t_fin", [CK, NHC * DPC]), ("t_enc", [CK, NHC * SB]),
                        ("t_xg", [128, NT * W]), ("t_xT", [CK, NWC * SB])):
            taps[nm] = nc.dram_tensor(nm, shp, f32, kind="ExternalOutput")
        taps["t_idx"] = nc.dram_tensor("t_idx", [128, NT], i32, kind="ExternalOutput")

    with tile.TileContext(nc) as tc:
        with (
            tc.tile_pool(name="sb", bufs=1) as pool,
            tc.tile_pool(name="ps", bufs=7, space="PSUM") as psum,
            tc.tile_pool(name="wu", bufs=1, space="PSUM") as wupool,
        ):
            mm = nc.tensor.matmul

            # --- index load + embedding gather first (gpsimd is dedicated
            # to the gather; everything else goes over HWDGE) -------------
            idxt = pool.tile([128, NT], i32, tag="idxt")
            nc.sync.dma_start(idxt[:], idx_d[:])
            # NOTE: one indirect DMA per 128 rows — the HW SWDGE gather only
            # supports a single [128, 1] offset column per instruction (a
            # batched [128, NT] offset AP simulates fine but gathers garbage
            # on hardware).
            xg_all = pool.tile([128, NT, W], f32, tag="xg")
            for t in range(NT):
                nc.gpsimd.indirect_dma_start(
                    out=xg_all[:, t, :], out_offset=None, in_=E_d[:],
                    in_offset=bass.IndirectOffsetOnAxis(ap=idxt[:, t:t + 1], axis=0),
                )
            xg = [xg_all[:, t, :] for t in range(NT)]

            # --- constants / weights in (HWDGE), earliest-needed first --
            aux = pool.tile([128, 258], f32, tag="aux")
            nc.sync.dma_start(aux[:], aux_d[:])
            ident = aux[:, 0:128]
            ones_c = aux[0:S, 128:129]
            ones_r = aux[0:1, 129:257]
            ones_c128 = aux[:, 128:129]
            zrow = aux[0:1, 64:128]  # identity row 0, upper half: all zeros
            ws = pool.tile([CK, NWC, H], fmm, tag="ws")
            wp = pool.tile([CK, NHC, H], fmm, tag="wp")
            wc = pool.tile([CK, NHC, H], fmm, tag="wc")
            wr = pool.tile([CK, NKR, H], fmm, tag="wr")
            nc.sync.dma_start(ws[:], wsT_d[:].rearrange("(c p) h -> p c h", p=CK))
            ball = pool.tile([CK, BALL_COLS], f32, tag="ball")
            nc.sync.dma_start(ball[:], ball_d[:])
            wroot = pool.tile([CK, NHC], fmm, tag="wroot")
            nc.sync.dma_start(wroot[:], wrc_d[:])
            maskt = pool.tile([S, SB], f32, tag="maskt")
            nc.sync.dma_start(maskt[:], mask_d[:])
            nc.sync.dma_start(wp[:], wpT_d[:].rearrange("(c p) h -> p c h", p=CK))
            nc.sync.dma_start(wc[:], wcT_d[:].rearrange("(c p) h -> p c h", p=CK))
            rootr = pool.tile([1, H], fmm, tag="rootr")
            nc.sync.dma_start(rootr[:], root_d[:])
            nc.sync.dma_start(wr[:], wrT_d[:].rearrange("(c p) h -> p c h", p=CK))
            ones_rr = pool.tile([1, 128], fmm, tag="ones_rr")
            nc.sync.dma_start(ones_rr[:], auxr_d[:])

            # --- PE warm-up: ~3.5us of junk transposes while the gather
            # is in flight, so HAM reaches 2.4 GHz before the real matmuls.
            wut = wupool.tile([128, 128], f32, tag="wut")

            def pe_filler(n):
                # Real matmuls (transpose-mode doesn't register as PE-busy
                # for the HAM clock gate): keep the PE warm across known
                # sem-wait windows.
                for _ in range(n):
                    mm(wut[:], ident, ident)

            pe_filler(16)

            # --- xT[w, s_g] via PE transpose ----------------------------
            xT = pool.tile([CK, NWC, SB], fmm, tag="xT")
            for w_i in range(NWC):
                xp = psum.tile([CK, SB], f32, tag="ps")
                for t in range(NT):
                    nc.tensor.transpose(
                        xp[:, 128 * t:128 * (t + 1)],
                        xg[t][:, CK * w_i:CK * (w_i + 1)], ident)
                nc.vector.tensor_copy(xT[:, w_i, :], xp[:])

            # --- encT = lrelu(W_sent @ x^T + b_sent) --------------------
            encT = pool.tile([CK, NHC, SB], fmm, tag="encT")
            for hc in range(NHC):
                pe = psum.tile([CK, SB], f32, tag="ps")
                for w_i in range(NWC):
                    mm(pe[:], ws[:, w_i, CK * hc:CK * (hc + 1)],
                       xT[:, w_i, :],
                       start=(w_i == 0), stop=(w_i == NWC - 1))
                nc.scalar.activation(encT[:, hc, :], pe[:], AF.Lrelu,
                                     bias=ball[:, 3 + hc:4 + hc], alpha=0.01)

            # --- P^T, C^T -----------------------------------------------
            PT = pool.tile([CK, NHC, SB], fmm, tag="PT")
            CT = pool.tile([CK, NHC, SB], fmm, tag="CT")
            for dst, wgt, bc in ((PT, wp, 6), (CT, wc, 9)):
                for hc in range(NHC):
                    pp = psum.tile([CK, SB], f32, tag="ps")
                    for kc in range(NHC):
                        mm(pp[:], wgt[:, kc, CK * hc:CK * (hc + 1)],
                           encT[:, kc, :],
                           start=(kc == 0), stop=(kc == NHC - 1))
                    nc.scalar.activation(dst[:, hc, :], pp[:], AF.Lrelu,
                                         bias=ball[:, bc + hc:bc + hc + 1],
                                         alpha=0.01)

            # --- root scores -> fri (early: needs only encT) ------------
            psc = psum.tile([1, SB], f32, tag="ps")
            for hc in range(NHC):
                mm(psc[:], wroot[:, hc:hc + 1], encT[:, hc, :],
                   start=(hc == 0), stop=(hc == NHC - 1))
            e_fri = pool.tile([1, SB], f32, tag="e_fri")
            nc.scalar.activation(e_fri[:], psc[:], AF.Exp)
            sfri = pool.tile([1, DPC], f32, tag="sfri")
            nc.vector.reduce_sum(sfri[:], e_fri[:].rearrange("p (d s) -> p d s", d=DPC),
                                 axis=X)
            rfri = pool.tile([1, DPC], f32, tag="rfri")
            nc.vector.reciprocal(rfri[:], sfri[:])
            fri_sb = pool.tile([1, SB], f32, tag="fri_sb")
            nc.vector.tensor_tensor(
                fri_sb[:].rearrange("p (d s) -> p d s", d=DPC),
                e_fri[:].rearrange("p (d s) -> p d s", d=DPC),
                rfri[:].rearrange("p (d o) -> p d o", o=1).to_broadcast([1, DPC, S]),
                op=mybir.AluOpType.mult)
            nc.sync.dma_start(fri_out[:], fri_sb[:])
            fri_r = pool.tile([1, SB], fmm, tag="fri_r")
            nc.vector.tensor_copy(fri_r[:], fri_sb[:])

            # --- riT accumulators: the encT contribution can run early,
            # filling PE gaps while the A stage's ACT/DVE work runs -------
            cat0 = []
            riT_ps = []
            for gc in range(NHC):
                pr = psum.tile([CK, SB], f32, tag="ps")
                riT_ps.append(pr)
                for kc in range(NHC):
                    mm(pr[:], wr[:, kc, CK * gc:CK * (gc + 1)], encT[:, kc, :],
                       start=(kc == 0), stop=False, skip_group_check=True)

            # --- enc in global [s_g, h] orientation (2 docs per tile) ---
            enc_g = pool.tile([128, NT, H], f32, tag="enc_g")
            for t in range(NT):
                ep = psum.tile([128, H], f32, tag="ps")
                for hc in range(NHC):
                    nc.tensor.transpose(
                        ep[:, CK * hc:CK * (hc + 1)],
                        encT[:, hc, 128 * t:128 * (t + 1)].bitcast(f32),
                        ident[:CK, :CK])
                nc.scalar.activation(enc_g[:, t, :], ep[:], AF.Copy)

            # Block-diagonal staging of A: Ablk[q, 128t+c] holds doc 2t
            # (rows 0:64) / doc 2t+1 (rows 64:128) so tmp becomes 4 dense
            # K=128 matmuls per h-chunk instead of 8 per-doc ones.
            Ablk = pool.tile([128, SB], f32, tag="Ablk")
            nc.vector.memset(Ablk[:], 0.0)

            # --- A: raw scores, exp, mask, column-normalize -------------
            pA = psum.tile([S, SB], f32, tag="ps")
            for d in range(DPC):
                sl = slice(S * d, S * (d + 1))
                for hc in range(NHC):
                    mm(pA[:, sl], PT[:, hc, sl], CT[:, hc, sl],
                       start=(hc == 0), stop=(hc == NHC - 1))
            pe_filler(8)
            e_raw = pool.tile([S, SB], f32, tag="e_raw")
            nc.scalar.activation(e_raw[:], pA[:], AF.Exp)
            e_m = pool.tile([S, SB], f32, tag="e_m")
            nc.vector.tensor_mul(e_m[:], e_raw[:], maskt[:])
            # Prefetch the odd-doc band (partition shift of 64) with the
            # *unnormalized* values so the DMA overlaps the sums/reciprocal;
            # the odd band is normalized in place below.
            Ablk_v = Ablk[:].rearrange("p (t c) -> p t c", t=NT)
            e_v = e_m[:].rearrange("p (t c) -> p t c", t=NT)
            nc.sync.dma_start(Ablk_v[S:128, :, S:128], e_v[:, :, S:128])
            psS = psum.tile([1, SB], f32, tag="ps")
            mm(psS[:], ones_c, e_m[:])
            recip = pool.tile([1, SB], f32, tag="recip")
            nc.vector.reciprocal_approx_fast(out=recip[:], in_=psS[:])
            psB = psum.tile([128, SB], f32, tag="ps")
            mm(psB[:], ones_r[:, 0:128], recip[:])
            pe_filler(8)
            psB_v = psB[:].rearrange("p (t c) -> p t c", t=NT)
            A_sb = pool.tile([S, SB], f32, tag="A_sb")
            nc.vector.tensor_mul(A_sb[:], e_m[:], psB[0:S, :])
            nc.sync.dma_start(A_out[:], A_sb[:])
            nc.vector.tensor_mul(Ablk_v[S:128, :, S:128],
                                 Ablk_v[S:128, :, S:128], psB_v[S:128, :, S:128])
            A_v = A_sb[:].rearrange("p (t c) -> p t c", t=NT)
            nc.vector.tensor_copy(Ablk_v[0:S, :, 0:S], A_v[:, :, 0:S])

            # --- row sums of A -> row vector in (d, i) order ------------
            sumA = pool.tile([S, DPC], f32, tag="sumA")
            nc.vector.reduce_sum(sumA[:], A_sb[:].rearrange("p (d j) -> p d j", d=DPC),
                                 axis=X)
            psT = psum.tile([DPC, S], f32, tag="ps")
            nc.tensor.transpose(psT[:], sumA[:], ident[:S, :S])
            sumAT = pool.tile([DPC, S], f32, tag="sumAT")
            nc.vector.tensor_copy(sumAT[:], psT[:])
            sumRow = pool.tile([1, SB], f32, tag="sumRow")
            nc.sync.dma_start(sumRow[:], sumAT[:])
            sumRow_r = pool.tile([1, SB], fmm, tag="sumRow_r")
            nc.vector.tensor_copy(sumRow_r[:], sumRow[:])

            # --- aggT = enc^T A (block-diag) + root x fri ---------------
            aggT = pool.tile([CK, NHC, SB], fmm, tag="aggT")
            for hc in range(NHC):
                pt = psum.tile([CK, SB], f32, tag="ps")
                # Outer product first: start=True sets has_written for the
                # whole bank, so the matmuls below accumulate onto it.
                mm(pt[:], rootr[:, CK * hc:CK * (hc + 1)], fri_r[:],
                   start=True, stop=False, skip_group_check=True)
                for t in range(NT):
                    sl = slice(128 * t, 128 * (t + 1))
                    mm(pt[:, sl], enc_g[:, t, CK * hc:CK * (hc + 1)], Ablk[:, sl],
                       start=False, stop=(t == NT - 1), skip_group_check=True)
                nc.scalar.activation(aggT[:, hc, :], pt[:], AF.Copy)

            # --- tmp3T = encT * broadcast(sumRow) -----------------------
            ps3 = psum.tile([CK, SB], f32, tag="ps")
            mm(ps3[:], ones_rr[:, :CK], sumRow_r[:])
            tmp3T = pool.tile([CK, NHC, SB], fmm, tag="tmp3T")
            for hc in range(NHC):
                nc.vector.tensor_mul(tmp3T[:, hc, :], encT[:, hc, :], ps3[:])

            pe_filler(6)

            # --- riT: remaining chunks, then lrelu on DVE (no ACT
            # table switch away from Exp) --------------------------------
            cat = [encT, aggT, tmp3T]
            riT = pool.tile([CK, NHC, SB], f32, tag="riT")
            for gc in range(NHC):
                pr = riT_ps[gc]
                for kc in range(NHC, NKR):
                    src = cat[kc // 3][:, kc % 3, :]
                    mm(pr[:], wr[:, kc, CK * gc:CK * (gc + 1)], src,
                       start=False, stop=(kc == NKR - 1), skip_group_check=True)
                nc.scalar.activation(riT[:, gc, :], pr[:], AF.Lrelu,
                                     bias=ball[:, 12 + gc:13 + gc], alpha=0.01)

            # --- final mean + classifier --------------------------------
            fin = pool.tile([CK, NHC, DPC], f32, tag="fin")
            for gc in range(NHC):
                nc.vector.reduce_sum(
                    fin[:, gc, :],
                    riT[:, gc, :].rearrange("p (d s) -> p d s", d=DPC), axis=X)
            pe_filler(8)
            pc = psum.tile([2, DPC], f32, tag="ps")
            for gc in range(NHC):
                mm(pc[:], ball[:, 15 + 2 * gc:17 + 2 * gc], fin[:, gc, :],
                   start=(gc == 0), stop=(gc == NHC - 1))
            cls_sb = pool.tile([2, DPC], f32, tag="cls_sb")
            nc.vector.tensor_scalar(
                out=cls_sb[:], in0=pc[:], scalar1=1.0 / S,
                scalar2=ball[0:2, 21:22],
                op0=mybir.AluOpType.mult, op1=mybir.AluOpType.add)
            nc.sync.dma_start(cls_out[:], cls_sb[:])
            if debug_taps:
                nc.sync.dma_start(taps["t_wr"][:], wr[:].bitcast(f32))
                nc.sync.dma_start(taps["t_enco"][:], enc_g[:].rearrange("p t h -> p (t h)"))
                nc.sync.dma_start(taps["t_sumrow"][:], sumRow[:])
                nc.sync.dma_start(taps["t_agg"][:], aggT[:].bitcast(f32))
                nc.sync.dma_start(taps["t_tmp3"][:], tmp3T[:].bitcast(f32))
                nc.sync.dma_start(taps["t_ri"][:], riT[:])
                nc.sync.dma_start(taps["t_fin"][:], fin[:])
                nc.sync.dma_start(taps["t_enc"][:], encT[:].bitcast(f32))
                nc.sync.dma_start(taps["t_xg"][:], xg_all[:])
                nc.sync.dma_start(taps["t_xT"][:], xT[:].bitcast(f32))
                nc.sync.dma_start(taps["t_idx"][:], idxt[:])

    nc.compile()
    return nc


def get_program(use_f32r=None, debug_taps=False):
    if use_f32r is None:
        use_f32r = USE_F32R
    key = (use_f32r, debug_taps)
    if key not in _PROGRAM_CACHE:
        _PROGRAM_CACHE[key] = _build_program(use_f32r, debug_taps)
    return _PROGRAM_CACHE[key]


def prep_inputs(word_indices, E, W_sent, b_sent, W_par, b_par, W_ch, b_ch,
                w_root, b_root, root_embed, W_r, b_r, W_cls, b_cls):
    """Shard + reformat the full inputs into per-core in_maps."""
    f = lambda a: np.ascontiguousarray(np.asarray(a), dtype=np.float32)
    idx_last = np.asarray(word_indices)[:, :, L - 1].astype(np.int32)  # [D, S]

    E = f(E)
    wsT = f(np.asarray(W_sent).T)
    wpT = f(np.asarray(W_par).T)
    wcT = f(np.asarray(W_ch).T)
    wrT = f(np.asarray(W_r).T)

    ball = np.zeros((CK, BALL_COLS), np.float32)
    w_root, b_sent, b_par, b_ch, b_r = map(f, (w_root, b_sent, b_par, b_ch, b_r))
    W_cls, b_cls = f(W_cls), f(b_cls)
    for c in range(NHC):
        sl = slice(CK * c, CK * (c + 1))
        ball[:, 0 + c] = w_root[sl]
        ball[:, 3 + c] = b_sent[sl]
        ball[:, 6 + c] = b_par[sl]
        ball[:, 9 + c] = b_ch[sl]
        ball[:, 12 + c] = b_r[sl]
        ball[:, 15 + 2 * c:17 + 2 * c] = W_cls[:, sl].T
    ball[0:2, 21] = b_cls

    rootr = f(root_embed).reshape(1, H)
    wrootc = np.stack([w_root[CK * c:CK * (c + 1)] for c in range(NHC)], axis=1)
    wrootc = np.ascontiguousarray(wrootc)

    mask = np.ones((S, SB), np.float32)
    for i in range(S):
        mask[i, i::S] = 0.0

    aux = np.zeros((128, 258), np.float32)
    aux[:, 0:128] = np.eye(128, dtype=np.float32)
    aux[:, 128] = 1.0
    aux[0, 129:257] = 1.0
    auxr = np.ones((1, 128), np.float32)

    in_maps = []
    for c in range(NCORES):
        flat = idx_last[DPC * c:DPC * (c + 1)].reshape(-1)        # (d, s) order
        idx_pm = flat.reshape(SB // 128, 128).T.copy()            # [128, NT]
        in_maps.append({
            "idx": idx_pm, "E": E, "wsT": wsT, "wpT": wpT, "wcT": wcT,
            "wrT": wrT, "wrootc": wrootc, "ball": ball, "rootr": rootr,
            "mask": mask, "aux": aux, "auxr": auxr,
        })
    return in_maps


def unshard(results):
    outs, As, fris = [], [], []
    for r in results:
        outs.append(r["clsout"].T)          # [DPC, 2]
        # Aout is [i, (d, j)] on device; reorder to [d, i, j]
        As.append(r["Aout"].reshape(S, DPC, S).transpose(1, 0, 2))
        fris.append(r["friout"])            # [DPC, S]
    return (np.concatenate(outs, axis=0).astype(np.float32),
            np.concatenate(As, axis=0).astype(np.float32),
            np.concatenate(fris, axis=0).astype(np.float32))


def kernel(**inputs):
    global LAST_RESULT
    from concourse.bass_utils import run_bass_kernel_spmd

    nc = get_program()
    in_maps = prep_inputs(**inputs)
    res = run_bass_kernel_spmd(nc, in_maps, core_ids=list(range(NCORES)),
                               trace=TRACE, **TRACE_KW)
    LAST_RESULT = res
    return unshard(res.results)


# revision 26
# speedup vs baseline: 1.1157x; 1.0686x over previous
"""Trainium2 Bass kernel for nn_DependencyBLSTM (gnn_message_passing).

Reference computation (per doc d, S=64 sentences):
    x      = E[word_indices[:, :, -1]]            # only the last word is used
    enc    = lrelu(x @ W_sent.T + b_sent)
    P      = lrelu(enc @ W_par.T + b_par)
    C      = lrelu(enc @ W_ch.T + b_ch)
    A      = softmax_over_i(mask_diag(P @ C.T))   # [S, S], softmax over axis i
    fri    = softmax(enc @ w_root + b_root)
    tmp    = A.T @ enc
    agg    = tmp + fri[:, None] * root_embed
    tmp3   = enc * A.sum(axis=1)[:, None]
    ri     = lrelu(concat([enc, agg, tmp3]) @ W_r.T + b_r)
    out    = ri.mean(axis=0) @ W_cls.T + b_cls
Returns (out [D,2], A [D,S,S], fri [D,S]).

Sharding: data-parallel over docs, 8 docs per core (512 sentences per core).
Embedding table replicated in device DRAM; rows gathered by indirect DMA.

On-chip layout (per core): feature dims (W=H=300) are chunked 3x100 on the
partition axis; the 512-sentence batch lives on the free axis (matches the
512-element fp32 PSUM bank / matmul moving-operand limit). The softmax over
parents (a partition-axis reduction) is done on the PE: ones-matmul column
sum, reciprocal on DVE, then a K=1 ones-outer-product matmul to broadcast
back across partitions.
"""

import numpy as np

D, S, L, W, H, V = 64, 64, 64, 300, 300, 50000
NCORES = 8
DPC = D // NCORES          # docs per core
SB = DPC * S               # sentence batch per core = 512
CK = 100                   # feature chunk size
NWC = W // CK              # 3 chunks over W
NHC = H // CK              # 3 chunks over H
NKR = 3 * H // CK          # 9 chunks over the concat feature dim

# b_all column map (packed per-partition constants, [CK, 22]):
#   0..2   w_root chunks          3..5   b_sent chunks
#   6..8   b_par chunks           9..11  b_ch chunks
#   12..14 b_r chunks             15..20 W_cls.T chunks ([CK,2] each)
#   21     b_cls (partitions 0..1)
BALL_COLS = 22

# Set by test.py for profiling; harness uses defaults.
TRACE = False
TRACE_KW = {}
LAST_RESULT = None
USE_F32R = True

_PROGRAM_CACHE = {}


def _build_program(use_f32r, debug_taps=False):
    import concourse.bacc as bacc
    import concourse.bass as bass
    import concourse.mybir as mybir
    import concourse.tile as tile

    f32 = mybir.dt.float32
    i32 = mybir.dt.int32
    AF = mybir.ActivationFunctionType
    X = mybir.AxisListType.X

    nc = bacc.Bacc("TRN2", target_bir_lowering=False, debug=False,
                   num_devices=NCORES)

    # --- DRAM I/O -------------------------------------------------------
    # fp32r (reduced-precision fp32) runs the big matmuls at 4x the fp32
    # rate when the moving dim is >= 256. Walrus requires every operand of
    # an fp32r matmul to be produced as fp32r, so weight DRAM tensors are
    # declared fp32r (bitwise-identical layout) and intermediate operand
    # tiles get dtype float32r with their producers casting on write.
    fmm = mybir.dt.float32r if use_f32r else f32
    NT = SB // 128  # gather tiles

    idx_d = nc.dram_tensor("idx", [128, SB // 128], i32, kind="ExternalInput")
    E_d = nc.dram_tensor("E", [V, W], f32, kind="ExternalInput")
    wsT_d = nc.dram_tensor("wsT", [W, H], fmm, kind="ExternalInput")
    wpT_d = nc.dram_tensor("wpT", [H, H], fmm, kind="ExternalInput")
    wcT_d = nc.dram_tensor("wcT", [H, H], fmm, kind="ExternalInput")
    wrT_d = nc.dram_tensor("wrT", [3 * H, H], fmm, kind="ExternalInput")
    wrc_d = nc.dram_tensor("wrootc", [CK, NHC], fmm, kind="ExternalInput")
    ball_d = nc.dram_tensor("ball", [CK, BALL_COLS], f32, kind="ExternalInput")
    root_d = nc.dram_tensor("rootr", [1, H], fmm, kind="ExternalInput")
    mask_d = nc.dram_tensor("mask", [S, SB], f32, kind="ExternalInput")
    aux_d = nc.dram_tensor("aux", [128, 258], f32, kind="ExternalInput")
    auxr_d = nc.dram_tensor("auxr", [1, 128], fmm, kind="ExternalInput")
    A_out = nc.dram_tensor("Aout", [S, SB], f32, kind="ExternalOutput")
    fri_out = nc.dram_tensor("friout", [DPC, S], f32, kind="ExternalOutput")
    cls_out = nc.dram_tensor("clsout", [2, DPC], f32, kind="ExternalOutput")
    taps = {}
    if debug_taps:
        for nm, shp in (("t_wr", [CK, NKR * H]), ("t_enco", [128, NT * H]),
                        ("t_sumrow", [1, SB]), ("t_agg", [CK, NHC * SB]),
                        ("t_tmp3", [CK, NHC * SB]), ("t_ri", [CK, NHC * SB]),
                        ("t_fin", [CK, NHC * DPC]), ("t_enc", [CK, NHC * SB]),
                        ("t_xg", [128, NT * W]), ("t_xT", [CK, NWC * SB])):
            taps[nm] = nc.dram_tensor(nm, shp, f32, kind="ExternalOutput")
        taps["t_idx"] = nc.dram_tensor("t_idx", [128, NT], i32, kind="ExternalOutput")

    with tile.TileContext(nc) as tc:
        with (
            tc.tile_pool(name="sb", bufs=1) as pool,
            tc.tile_pool(name="ps", bufs=7, space="PSUM") as psum,
            tc.tile_pool(name="wu", bufs=1, space="PSUM") as wupool,
        ):
            mm = nc.tensor.matmul

            # --- index load + embedding gather first (gpsimd is dedicated
            # to the gather; everything else goes over HWDGE) -------------
            idxt = pool.tile([128, NT], i32, tag="idxt")
            nc.sync.dma_start(idxt[:], idx_d[:])
            # NOTE: one indirect DMA per 128 rows — the HW SWDGE gather only
            # supports a single [128, 1] offset column per instruction (a
            # batched [128, NT] offset AP simulates fine but gathers garbage
            # on hardware).
            xg_all = pool.tile([128, NT, W], f32, tag="xg")
            for t in range(NT):
                nc.gpsimd.indirect_dma_start(
                    out=xg_all[:, t, :], out_offset=None, in_=E_d[:],
                    in_offset=bass.IndirectOffsetOnAxis(ap=idxt[:, t:t + 1], axis=0),
                )
            xg = [xg_all[:, t, :] for t in range(NT)]

            # --- constants / weights in (HWDGE), earliest-needed first --
            aux = pool.tile([128, 258], f32, tag="aux")
            nc.sync.dma_start(aux[:], aux_d[:])
            ident = aux[:, 0:128]
            ones_c = aux[0:S, 128:129]
            ones_r = aux[0:1, 129:257]
            ws = pool.tile([CK, NWC, H], fmm, tag="ws")
            wp = pool.tile([CK, NHC, H], fmm, tag="wp")
            wc = pool.tile([CK, NHC, H], fmm, tag="wc")
            wr = pool.tile([CK, NKR, H], fmm, tag="wr")
            nc.sync.dma_start(ws[:], wsT_d[:].rearrange("(c p) h -> p c h", p=CK))
            ball = pool.tile([CK, BALL_COLS], f32, tag="ball")
            nc.sync.dma_start(ball[:], ball_d[:])
            wroot = pool.tile([CK, NHC], fmm, tag="wroot")
            nc.sync.dma_start(wroot[:], wrc_d[:])
            maskt = pool.tile([S, SB], f32, tag="maskt")
            nc.sync.dma_start(maskt[:], mask_d[:])
            nc.sync.dma_start(wp[:], wpT_d[:].rearrange("(c p) h -> p c h", p=CK))
            nc.sync.dma_start(wc[:], wcT_d[:].rearrange("(c p) h -> p c h", p=CK))
            rootr = pool.tile([1, H], fmm, tag="rootr")
            nc.sync.dma_start(rootr[:], root_d[:])
            nc.sync.dma_start(wr[:], wrT_d[:].rearrange("(c p) h -> p c h", p=CK))
            ones_rr = pool.tile([1, 128], fmm, tag="ones_rr")
            nc.sync.dma_start(ones_rr[:], auxr_d[:])

            # --- PE warm-up: ~3.5us of junk transposes while the gather
            # is in flight, so HAM reaches 2.4 GHz before the real matmuls.
            wut = wupool.tile([128, 128], f32, tag="wut")

            def pe_filler(n):
                # Real matmuls (transpose-mode doesn't register as PE-busy
                # for the HAM clock gate): keep the PE warm across known
                # sem-wait windows.
                for _ in range(n):
                    mm(wut[:], ident, ident)

            pe_filler(16)

            # --- xT[w, s_g] via PE transpose ----------------------------
            xT = pool.tile([CK, NWC, SB], fmm, tag="xT")
            for w_i in range(NWC):
                xp = psum.tile([CK, SB], f32, tag="ps")
                for t in range(NT):
                    nc.tensor.transpose(
                        xp[:, 128 * t:128 * (t + 1)],
                        xg[t][:, CK * w_i:CK * (w_i + 1)], ident)
                nc.vector.tensor_copy(xT[:, w_i, :], xp[:])

            # --- encT = lrelu(W_sent @ x^T + b_sent) --------------------
            encT = pool.tile([CK, NHC, SB], fmm, tag="encT")
            for hc in range(NHC):
                pe = psum.tile([CK, SB], f32, tag="ps")
                for w_i in range(NWC):
                    mm(pe[:], ws[:, w_i, CK * hc:CK * (hc + 1)],
                       xT[:, w_i, :],
                       start=(w_i == 0), stop=(w_i == NWC - 1))
                nc.scalar.activation(encT[:, hc, :], pe[:], AF.Lrelu,
                                     bias=ball[:, 3 + hc:4 + hc], alpha=0.01)

            # --- P^T, C^T -----------------------------------------------
            PT = pool.tile([CK, NHC, SB], fmm, tag="PT")
            CT = pool.tile([CK, NHC, SB], fmm, tag="CT")
            for dst, wgt, bc in ((PT, wp, 6), (CT, wc, 9)):
                for hc in range(NHC):
                    pp = psum.tile([CK, SB], f32, tag="ps")
                    for kc in range(NHC):
                        mm(pp[:], wgt[:, kc, CK * hc:CK * (hc + 1)],
                           encT[:, kc, :],
                           start=(kc == 0), stop=(kc == NHC - 1))
                    nc.scalar.activation(dst[:, hc, :], pp[:], AF.Lrelu,
                                         bias=ball[:, bc + hc:bc + hc + 1],
                                         alpha=0.01)

            # --- root scores -> fri (early: needs only encT) ------------
            psc = psum.tile([1, SB], f32, tag="ps")
            for hc in range(NHC):
                mm(psc[:], wroot[:, hc:hc + 1], encT[:, hc, :],
                   start=(hc == 0), stop=(hc == NHC - 1))
            e_fri = pool.tile([1, SB], f32, tag="e_fri")
            nc.scalar.activation(e_fri[:], psc[:], AF.Exp)
            sfri = pool.tile([1, DPC], f32, tag="sfri")
            nc.vector.reduce_sum(sfri[:], e_fri[:].rearrange("p (d s) -> p d s", d=DPC),
                                 axis=X)
            rfri = pool.tile([1, DPC], f32, tag="rfri")
            nc.vector.reciprocal(rfri[:], sfri[:])
            fri_sb = pool.tile([1, SB], f32, tag="fri_sb")
            nc.vector.tensor_tensor(
                fri_sb[:].rearrange("p (d s) -> p d s", d=DPC),
                e_fri[:].rearrange("p (d s) -> p d s", d=DPC),
                rfri[:].rearrange("p (d o) -> p d o", o=1).to_broadcast([1, DPC, S]),
                op=mybir.AluOpType.mult)
            nc.sync.dma_start(fri_out[:], fri_sb[:])
            fri_r = pool.tile([1, SB], fmm, tag="fri_r")
            nc.vector.tensor_copy(fri_r[:], fri_sb[:])

            # --- riT accumulators: the encT contribution can run early,
            # filling PE gaps while the A stage's ACT/DVE work runs -------
            riT_ps = []
            for gc in range(NHC):
                pr = psum.tile([CK, SB], f32, tag="ps")
                riT_ps.append(pr)
                for kc in range(NHC):
                    mm(pr[:], wr[:, kc, CK * gc:CK * (gc + 1)], encT[:, kc, :],
                       start=(kc == 0), stop=False, skip_group_check=True)

            # --- enc in global [s_g, h] orientation (2 docs per tile) ---
            enc_g = pool.tile([128, NT, H], f32, tag="enc_g")
            for t in range(NT):
                ep = psum.tile([128, H], f32, tag="ps")
                for hc in range(NHC):
                    nc.tensor.transpose(
                        ep[:, CK * hc:CK * (hc + 1)],
                        encT[:, hc, 128 * t:128 * (t + 1)].bitcast(f32),
                        ident[:CK, :CK])
                nc.scalar.activation(enc_g[:, t, :], ep[:], AF.Copy)

            # Block-diagonal staging of A: Ablk[q, 128t+c] holds doc 2t
            # (rows 0:64) / doc 2t+1 (rows 64:128) so tmp becomes 4 dense
            # K=128 matmuls per h-chunk instead of 8 per-doc ones.
            Ablk = pool.tile([128, SB], f32, tag="Ablk")
            nc.vector.memset(Ablk[:], 0.0)

            # --- A: raw scores, exp, mask, column-normalize -------------
            pA = psum.tile([S, SB], f32, tag="ps")
            for d in range(DPC):
                sl = slice(S * d, S * (d + 1))
                for hc in range(NHC):
                    mm(pA[:, sl], PT[:, hc, sl], CT[:, hc, sl],
                       start=(hc == 0), stop=(hc == NHC - 1))
            pe_filler(8)
            e_raw = pool.tile([S, SB], f32, tag="e_raw")
            nc.scalar.activation(e_raw[:], pA[:], AF.Exp)
            e_m = pool.tile([S, SB], f32, tag="e_m")
            nc.vector.tensor_mul(e_m[:], e_raw[:], maskt[:])
            # Prefetch the odd-doc band (partition shift of 64) with the
            # *unnormalized* values so the DMA overlaps the sums/reciprocal;
            # the odd band is normalized in place below.
            Ablk_v = Ablk[:].rearrange("p (t c) -> p t c", t=NT)
            e_v = e_m[:].rearrange("p (t c) -> p t c", t=NT)
            nc.sync.dma_start(Ablk_v[S:128, :, S:128], e_v[:, :, S:128])
            psS = psum.tile([1, SB], f32, tag="ps")
            mm(psS[:], ones_c, e_m[:])
            recip = pool.tile([1, SB], f32, tag="recip")
            nc.vector.reciprocal_approx_fast(out=recip[:], in_=psS[:])
            psB = psum.tile([128, SB], f32, tag="ps")
            mm(psB[:], ones_r[:, 0:128], recip[:])
            pe_filler(8)
            psB_v = psB[:].rearrange("p (t c) -> p t c", t=NT)
            A_sb = pool.tile([S, SB], f32, tag="A_sb")
            nc.vector.tensor_mul(A_sb[:], e_m[:], psB[0:S, :])
            nc.sync.dma_start(A_out[:], A_sb[:])
            nc.vector.tensor_mul(Ablk_v[S:128, :, S:128],
                                 Ablk_v[S:128, :, S:128], psB_v[S:128, :, S:128])
            A_v = A_sb[:].rearrange("p (t c) -> p t c", t=NT)
            nc.vector.tensor_copy(Ablk_v[0:S, :, 0:S], A_v[:, :, 0:S])

            # --- row sums of A -> row vector in (d, i) order ------------
            sumA = pool.tile([S, DPC], f32, tag="sumA")
            nc.vector.reduce_sum(sumA[:], A_sb[:].rearrange("p (d j) -> p d j", d=DPC),
                                 axis=X)
            psT = psum.tile([DPC, S], f32, tag="ps")
            nc.tensor.transpose(psT[:], sumA[:], ident[:S, :S])
            sumAT = pool.tile([DPC, S], f32, tag="sumAT")
            nc.vector.tensor_copy(sumAT[:], psT[:])
            sumRow = pool.tile([1, SB], f32, tag="sumRow")
            nc.sync.dma_start(sumRow[:], sumAT[:])
            sumRow_r = pool.tile([1, SB], fmm, tag="sumRow_r")
            nc.vector.tensor_copy(sumRow_r[:], sumRow[:])

            # --- aggT = enc^T A (block-diag) + root x fri ---------------
            aggT = pool.tile([CK, NHC, SB], fmm, tag="aggT")
            for hc in range(NHC):
                pt = psum.tile([CK, SB], f32, tag="ps")
                # Outer product first: start=True sets has_written for the
                # whole bank, so the matmuls below accumulate onto it.
                mm(pt[:], rootr[:, CK * hc:CK * (hc + 1)], fri_r[:],
                   start=True, stop=False, skip_group_check=True)
                for t in range(NT):
                    sl = slice(128 * t, 128 * (t + 1))
                    mm(pt[:, sl], enc_g[:, t, CK * hc:CK * (hc + 1)], Ablk[:, sl],
                       start=False, stop=(t == NT - 1), skip_group_check=True)
                nc.scalar.activation(aggT[:, hc, :], pt[:], AF.Copy)

            # --- tmp3T = encT * broadcast(sumRow) -----------------------
            ps3 = psum.tile([CK, SB], f32, tag="ps")
            mm(ps3[:], ones_rr[:, :CK], sumRow_r[:])
            tmp3T = pool.tile([CK, NHC, SB], fmm, tag="tmp3T")
            for hc in range(NHC):
                nc.vector.tensor_mul(tmp3T[:, hc, :], encT[:, hc, :], ps3[:])

            pe_filler(6)

            # --- riT: remaining chunks, then lrelu on DVE (no ACT
            # table switch away from Exp) --------------------------------
            cat = [encT, aggT, tmp3T]
            riT = pool.tile([CK, NHC, SB], f32, tag="riT")
            for gc in range(NHC):
                pr = riT_ps[gc]
                for kc in range(NHC, NKR):
                    src = cat[kc // 3][:, kc % 3, :]
                    mm(pr[:], wr[:, kc, CK * gc:CK * (gc + 1)], src,
                       start=False, stop=(kc == NKR - 1), skip_group_check=True)
                nc.scalar.activation(riT[:, gc, :], pr[:], AF.Lrelu,
                                     bias=ball[:, 12 + gc:13 + gc], alpha=0.01)

            # --- final mean + classifier --------------------------------
            fin = pool.tile([CK, NHC, DPC], f32, tag="fin")
            for gc in range(NHC):
                nc.vector.reduce_sum(
                    fin[:, gc, :],
                    riT[:, gc, :].rearrange("p (d s) -> p d s", d=DPC), axis=X)
            pe_filler(8)
            pc = psum.tile([2, DPC], f32, tag="ps")
            for gc in range(NHC):
                mm(pc[:], ball[:, 15 + 2 * gc:17 + 2 * gc], fin[:, gc, :],
                   start=(gc == 0), stop=(gc == NHC - 1))
            cls_sb = pool.tile([2, DPC], f32, tag="cls_sb")
            nc.vector.tensor_scalar(
                out=cls_sb[:], in0=pc[:], scalar1=1.0 / S,
                scalar2=ball[0:2, 21:22],
                op0=mybir.AluOpType.mult, op1=mybir.AluOpType.add)
            nc.sync.dma_start(cls_out[:], cls_sb[:])
            if debug_taps:
                nc.sync.dma_start(taps["t_wr"][:], wr[:].bitcast(f32))
                nc.sync.dma_start(taps["t_enco"][:], enc_g[:].rearrange("p t h -> p (t h)"))
                nc.sync.dma_start(taps["t_sumrow"][:], sumRow[:])
                nc.sync.dma_start(taps["t_agg"][:], aggT[:].bitcast(f32))
                nc.sync.dma_start(taps["t_tmp3"][:], tmp3T[:].bitcast(f32))
                nc.sync.dma_start(taps["t_ri"][:], riT[:])
                nc.sync.dma_start(taps["t_fin"][:], fin[:])
                nc.sync.dma_start(taps["t_enc"][:], encT[:].bitcast(f32))
                nc.sync.dma_start(taps["t_xg"][:], xg_all[:])
                nc.sync.dma_start(taps["t_xT"][:], xT[:].bitcast(f32))
                nc.sync.dma_start(taps["t_idx"][:], idxt[:])

    nc.compile()
    return nc


def get_program(use_f32r=None, debug_taps=False):
    if use_f32r is None:
        use_f32r = USE_F32R
    key = (use_f32r, debug_taps)
    if key not in _PROGRAM_CACHE:
        _PROGRAM_CACHE[key] = _build_program(use_f32r, debug_taps)
    return _PROGRAM_CACHE[key]


def prep_inputs(word_indices, E, W_sent, b_sent, W_par, b_par, W_ch, b_ch,
                w_root, b_root, root_embed, W_r, b_r, W_cls, b_cls):
    """Shard + reformat the full inputs into per-core in_maps."""
    f = lambda a: np.ascontiguousarray(np.asarray(a), dtype=np.float32)
    idx_last = np.asarray(word_indices)[:, :, L - 1].astype(np.int32)  # [D, S]

    E = f(E)
    wsT = f(np.asarray(W_sent).T)
    wpT = f(np.asarray(W_par).T)
    wcT = f(np.asarray(W_ch).T)
    wrT = f(np.asarray(W_r).T)

    ball = np.zeros((CK, BALL_COLS), np.float32)
    w_root, b_sent, b_par, b_ch, b_r = map(f, (w_root, b_sent, b_par, b_ch, b_r))
    W_cls, b_cls = f(W_cls), f(b_cls)
    for c in range(NHC):
        sl = slice(CK * c, CK * (c + 1))
        ball[:, 0 + c] = w_root[sl]
        ball[:, 3 + c] = b_sent[sl]
        ball[:, 6 + c] = b_par[sl]
        ball[:, 9 + c] = b_ch[sl]
        ball[:, 12 + c] = b_r[sl]
        ball[:, 15 + 2 * c:17 + 2 * c] = W_cls[:, sl].T
    ball[0:2, 21] = b_cls

    rootr = f(root_embed).reshape(1, H)
    wrootc = np.stack([w_root[CK * c:CK * (c + 1)] for c in range(NHC)], axis=1)
    wrootc = np.ascontiguousarray(wrootc)

    mask = np.ones((S, SB), np.float32)
    for i in range(S):
        mask[i, i::S] = 0.0

    aux = np.zeros((128, 258), np.float32)
    aux[:, 0:128] = np.eye(128, dtype=np.float32)
    aux[:, 128] = 1.0
    aux[0, 129:257] = 1.0
    auxr = np.ones((1, 128), np.float32)

    in_maps = []
    for c in range(NCORES):
        flat = idx_last[DPC * c:DPC * (c + 1)].reshape(-1)        # (d, s) order
        idx_pm = flat.reshape(SB // 128, 128).T.copy()            # [128, NT]
        in_maps.append({
            "idx": idx_pm, "E": E, "wsT": wsT, "wpT": wpT, "wcT": wcT,
            "wrT": wrT, "wrootc": wrootc, "ball": ball, "rootr": rootr,
            "mask": mask, "aux": aux, "auxr": auxr,
        })
    return in_maps


def unshard(results):
    outs, As, fris = [], [], []
    for r in results:
        outs.append(r["clsout"].T)          # [DPC, 2]
        # Aout is [i, (d, j)] on device; reorder to [d, i, j]
        As.append(r["Aout"].reshape(S, DPC, S).transpose(1, 0, 2))
        fris.append(r["friout"])            # [DPC, S]
    return (np.concatenate(outs, axis=0).astype(np.float32),
            np.concatenate(As, axis=0).astype(np.float32),
            np.concatenate(fris, axis=0).astype(np.float32))


def kernel(**inputs):
    global LAST_RESULT
    from concourse.bass_utils import run_bass_kernel_spmd

    nc = get_program()
    in_maps = prep_inputs(**inputs)
    res = run_bass_kernel_spmd(nc, in_maps, core_ids=list(range(NCORES)),
                               trace=TRACE, **TRACE_KW)
    LAST_RESULT = res
    return unshard(res.results)
